# revision 60
# baseline (speedup 1.0000x reference)
"""Trainium2 Bass kernel for nn_MultiHeadAttention (B=2, S=2048, DM=1024, H=8).

Sharding: data-parallel on batch x tensor-parallel on heads.
Core c in 0..7 handles batch b = c//4 and heads {2*(c%4), 2*(c%4)+1}.
Each core computes its two heads' full attention and the partial
out-projection (a 1024x2048 partial sum); the host adds the 4 partials
per batch (plus bo) and transposes back to (S, DOUT).

v2 design (from trace analysis of v1):
  - projections in fp8-e4m3 DoubleRow (K=256 per pass), weights host-scaled
    x32 to stay out of the subnormal range; evacuation rescales by 1/32
  - x shipped pre-packed [t, kcp, p, i, s] fp8 so every DMA is contiguous;
    V first, then Q, K so the PE never waits on the V tiles
  - attention per (half, j, h): score pairs land in one [128,2,512] PSUM
    tile, one exp ACTIVATE per pair (N=1024), one masked-multiply per pair
  - rowsums via 4-way column-tiled ones-matmuls (output partitions
    0/32/64/96 hold per-query sums) -> reciprocal on [128,8] after a DRAM
    gather, broadcast back, normalize
  - out-projection interleaved per half; bo added on the host
"""

import sys

sys.path.insert(0, "/opt/trn_rl_repo")

import numpy as np
import ml_dtypes

import concourse.bass as bass
import concourse.tile as tile
from concourse import bacc, mybir
from concourse.bass import ts, ds
from concourse.bass_utils import run_bass_kernel_spmd

BF16 = mybir.dt.bfloat16
F32 = mybir.dt.float32
FP8 = mybir.dt.float8e4
Copy = mybir.ActivationFunctionType.Copy
Exp = mybir.ActivationFunctionType.Exp
ADD = mybir.AluOpType.add
MULT = mybir.AluOpType.mult
DR = mybir.MatmulPerfMode.DoubleRow

B, S, DM, H, DOUT = 2, 2048, 1024, 8, 1024
D = DM // H            # 128 head dim
NH = 2                 # heads per core
KC = DM // 128         # 8 contraction chunks for projections
KCP = KC // 2          # 4 DoubleRow chunk pairs
OC = S // 128          # 16 key chunks
NT = 512
SCALE = float(1.0 / np.sqrt(np.float32(D)))

FP8_PROJ = False       # q/k/v projections in fp8 DoubleRow
FP8_OUTPROJ = False    # out-projection in fp8 DoubleRow
WSC = 32.0             # fp8 weight pre-scale (host); evac rescales by 1/WSC

XDT = FP8 if FP8_PROJ else BF16
ODT = FP8 if FP8_OUTPROJ else BF16


def build():
    nc = bacc.Bacc(None, target_bir_lowering=False)

    # x packed as chunk pairs [t, kcp, p, i, s] so each DMA is 1MB contiguous
    if FP8_PROJ:
        x_in = nc.dram_tensor("x", [3, KCP, 128, 2, S], FP8, kind="ExternalInput")
        w_in = nc.dram_tensor("w", [128, 3, KCP, 2, NH, D], FP8, kind="ExternalInput")
    else:
        x_in = nc.dram_tensor("x", [3, KCP, 128, 2, S], BF16, kind="ExternalInput")
        w_in = nc.dram_tensor("w", [128, 3, KC, NH, D], BF16, kind="ExternalInput")
    # mask packed per (half, j) quarter: [half, j, p, oc, 512]
    mask_in = nc.dram_tensor("maskP", [2, 2, 128, OC, NT], BF16, kind="ExternalInput")
    wo_in = nc.dram_tensor("wo", [D, NH, DOUT // 128, 128], ODT, kind="ExternalInput")
    ident_in = nc.dram_tensor("ident", [128, 128], BF16, kind="ExternalInput")
    outT = nc.dram_tensor("outT", [DOUT, S], F32, kind="ExternalOutput")

    qA, qB, qC = nc.sync, nc.gpsimd, nc.scalar

    with tile.TileContext(nc) as tc:
        with (
            tc.tile_pool(name="const", bufs=1) as constp,
            tc.tile_pool(name="xin", bufs=7) as xp,
            tc.tile_pool(name="mask", bufs=3) as mp,
            tc.tile_pool(name="pt", bufs=3) as pp,
            tc.tile_pool(name="rb", bufs=2) as rbp,
            tc.tile_pool(name="fout", bufs=6) as fop,
            tc.tile_pool(name="psum", bufs=2, space="PSUM") as psp,
            tc.tile_pool(name="dram", bufs=2, space="DRAM") as dramp,
        ):
            # ---- constants; w split per tensor, V's slice first (queue C) ----
            ident_sb = constp.tile([128, 128], BF16)
            qC.dma_start(out=ident_sb, in_=ident_in[:])
            if FP8_PROJ:
                w_sb = constp.tile([128, 3, KCP, 2, NH, D], FP8)
            else:
                w_sb = constp.tile([128, 3, KC, NH, D], BF16)
            for t in (2, 0, 1):
                if FP8_PROJ:
                    qC.dma_start(out=w_sb[:, t], in_=w_in[:, t])
                else:
                    # 128KB chunks so the first projection matmul isn't
                    # gated on the whole tensor's weights landing
                    for kcp in range(KCP):
                        qC.dma_start(
                            out=w_sb[:, t, ds(2 * kcp, 2)],
                            in_=w_in[:, t, ds(2 * kcp, 2)],
                        )
            ones_col = constp.tile([128, 1], BF16)
            nc.vector.memset(ones_col, 1.0)

            # ---- x tiles, V first then Q then K, alternating queues ----
            # t order: v(2), q(0), k(1)
            # bulk loads only on the two HWDGE queues (sync + scalar);
            # gpsimd's SWDGE is far slower for large transfers
            xts = {}
            qi = 0
            xdt = FP8 if FP8_PROJ else BF16
            for t in (2, 0, 1):
                for kcp in range(KCP):
                    xt = xp.tile([128, 2, S], xdt, tag="x", name=f"x{t}_{kcp}")
                    q = (qA, qB)[qi % 2]
                    if t == 2 and kcp < 2:
                        # split the first V tiles so the PE starts sooner
                        q.dma_start(out=xt[:, 0, :], in_=x_in[t, kcp, :, 0, :])
                        q.dma_start(out=xt[:, 1, :], in_=x_in[t, kcp, :, 1, :])
                    else:
                        q.dma_start(out=xt, in_=x_in[t, kcp])
                    xts[(t, kcp)] = xt
                    qi += 1

            # mask after x on the same queues, quarter tiles with 2-oc chunk
            # DMAs so the attention start isn't gated on a whole half landing
            mask_sb = {}
            for half in range(2):
                for j in range(2):
                    m = mp.tile([128, OC, NT], BF16, tag="mask", name=f"m{half}{j}")
                    for mc in range(4):
                        (qA, qB)[(half * 2 + j + mc) % 2].dma_start(
                            out=m[:, ds(4 * mc, 4), :],
                            in_=mask_in[half, j, :, ds(4 * mc, 4), :],
                        )
                    mask_sb[(half, j)] = m

            # wo only needed once attention output exists — keep it off the
            # critical preamble
            wo_sb = constp.tile([D, NH, DOUT // 128, 128], ODT)
            qC.dma_start(out=wo_sb, in_=wo_in[:])

            # ---- projections: vpt / qk ----
            qk_sb = constp.tile([128, 2, NH, S], BF16)   # [d, t(q/k), h, s]
            vpt_sb = constp.tile([128, NH, S], BF16)     # [d, h, s]
            vp_sb = constp.tile([128, OC, NH, D], BF16)  # [s%128, oc, h, d]

            for t in (2, 0, 1):
                for h in range(NH):
                    acc = [
                        psp.tile([128, 2, NT], F32, tag="w2", bufs=2, name=f"acc{i}")
                        for i in range(2)
                    ]
                    if FP8_PROJ:
                        for kcp in range(KCP):
                            for it in range(4):
                                nc.tensor.matmul(
                                    acc[it // 2][:, it % 2, :],
                                    w_sb[:, t, kcp, :, h, :],
                                    xts[(t, kcp)][:, :, ts(it, NT)],
                                    start=(kcp == 0),
                                    stop=(kcp == KCP - 1),
                                    perf_mode=DR,
                                )
                    else:
                        for kc in range(KC):
                            kcp, i = kc // 2, kc % 2
                            for it in range(4):
                                nc.tensor.matmul(
                                    acc[it // 2][:, it % 2, :],
                                    w_sb[:, t, kc, h, :],
                                    xts[(t, kcp)][:, i, ts(it, NT)],
                                    start=(kc == 0),
                                    stop=(kc == KC - 1),
                                )
                    # biases are zero in this model family; evac on the idle ACT
                    dst = vpt_sb[:, h, :] if t == 2 else qk_sb[:, t, h, :]
                    esc = (1.0 / WSC) if FP8_PROJ else 1.0
                    for it in range(4):
                        nc.scalar.activation(
                            out=dst[:, ts(it, NT)],
                            in_=acc[it // 2][:, it % 2, :],
                            func=Copy,
                            bias=0.0,
                            scale=esc,
                        )
                if t == 2:
                    # transpose Vp to natural layout while Q/K proj proceeds
                    for h in range(NH):
                        for oc in range(OC):
                            # share the attention-only outp slots: 2-wide
                            # rotation lets transpose(i+1) overlap copy(i)
                            tps = psp.tile(
                                [128, D], BF16, tag="outp", bufs=2, name="tps"
                            )
                            nc.tensor.transpose(
                                tps, vpt_sb[:, h, ds(oc * 128, 128)], ident_sb
                            )
                            nc.vector.tensor_copy(vp_sb[:, oc, h, :], tps)

            # ---- attention ----
            outn_sb = constp.tile([128, NH, S], ODT)  # normalized out, [d, h, s]

            for half in range(2):
                i0 = half * 2 * NT
                for j in range(2):
                    q0 = i0 + j * NT
                    # rowsums: 4-way col-tiled at rows 32*(2i+cg), heads in
                    # column halves; den[q] = i0-row + i1-row (summed on DVE
                    # after the reshape gather)
                    rp = psp.tile([128, NT], F32, tag="rp", bufs=1, name="rp")
                    osb = {}
                    for h in range(NH):
                        outp = psp.tile(
                            [128, NT], F32, tag="outp", bufs=2, name=f"outp{h}"
                        )
                        deferred = None
                        for m in range(OC // 2):
                            sps = psp.tile(
                                [128, 2, NT], F32, tag="w2", bufs=2, name="sps"
                            )
                            for i in range(2):
                                oc = 2 * m + i
                                nc.tensor.matmul(
                                    sps[:, i, :],
                                    qk_sb[:, 1, h, ds(oc * 128, 128)],
                                    qk_sb[:, 0, h, ds(q0, NT)],
                                    start=True,
                                    stop=True,
                                )
                            p = pp.tile([128, 2, NT], BF16, tag="p")
                            nc.scalar.activation(
                                out=p, in_=sps, func=Exp, bias=0.0, scale=SCALE
                            )
                            pm = pp.tile([128, 2, NT], BF16, tag="pm")
                            nc.vector.tensor_mul(
                                pm, p, mask_sb[(half, j)][:, ds(2 * m, 2), :]
                            )
                            if deferred is not None:
                                deferred()
                            def emit_out(m=m, pm=pm, h=h, outp=outp):
                                for i in range(2):
                                    oc = 2 * m + i
                                    nc.tensor.matmul(
                                        outp,
                                        vp_sb[:, oc, h, :],
                                        pm[:, i, :],
                                        start=(oc == 0),
                                        stop=(oc == OC - 1),
                                    )
                                for i in range(2):
                                    for cg in range(2):
                                        rb = 32 * (2 * i + cg)
                                        nc.tensor.matmul(
                                            rp[rb : rb + 1, ds(256 * h, 256)],
                                            ones_col,
                                            pm[:, i, ds(cg * 256, 256)],
                                            start=(m == 0),
                                            stop=(m == OC // 2 - 1),
                                            tile_position=(0, rb),
                                        )
                            deferred = emit_out
                        deferred()
                        # decouple the accumulator from the chain latency
                        o = rbp.tile([128, NT], F32, tag="osb", bufs=2, name=f"o{h}")
                        nc.vector.tensor_copy(o, outp)
                        osb[h] = o

                    # rowsum -> reciprocal chain (both heads at once); one
                    # whole-tile evac, the spare rows are unused garbage
                    r2 = rbp.tile([128, NT], F32, tag="r2")
                    nc.vector.tensor_copy(r2, rp)
                    # chain hops stay off qB so they never queue behind the
                    # 256KB fout output transfers
                    rd = dramp.tile([4, NT], F32, tag="rd")
                    for r in range(4):
                        (qA, qC)[r % 2].dma_start(
                            out=rd[r : r + 1, :], in_=r2[32 * r : 32 * r + 1, :]
                        )
                    rseg16 = rbp.tile([128, 16], F32, tag="rseg")
                    for i in range(2):
                        for h in range(2):
                            (qA, qC)[(2 * i + h) % 2].dma_start(
                                out=rseg16[ds(64 * h, 64), ts(i, 8)],
                                in_=rd[ds(2 * i, 2), ds(256 * h, 256)],
                            )
                    rseg = rbp.tile([128, 8], F32, tag="rseg8")
                    nc.vector.tensor_add(rseg, rseg16[:, 0:8], rseg16[:, 8:16])
                    nc.vector.reciprocal(rseg, rseg)
                    if FP8_OUTPROJ:
                        nc.vector.tensor_scalar_mul(rseg, rseg, WSC)
                    rd2 = dramp.tile([2, NT], F32, tag="rd2")
                    qA.dma_start(
                        out=rd2[:].rearrange("a b -> (a b)").rearrange(
                            "(p c) -> p c", p=128
                        ),
                        in_=rseg,
                    )
                    for h in range(NH):
                        rbc = rbp.tile([128, NT], F32, tag="rbc", bufs=2)
                        qA.dma_start(
                            out=rbc,
                            in_=rd2[h : h + 1, :].to_broadcast([128, NT]),
                        )
                        nc.vector.tensor_mul(
                            outn_sb[:, h, ds(q0, NT)], osb[h], rbc
                        )

                    # ---- out-projection for these 512 queries ----
                    # the final quarter has no attention left to overlap, so
                    # borrow the dead sps slots for 2-wide facc rotation
                    last = half == 1 and j == 1
                    for dc in range(DOUT // 128):
                        if last:
                            facc = psp.tile(
                                [128, 2, NT], F32, tag="w2", bufs=2, name="faccw"
                            )[:, 0, :]
                        else:
                            facc = psp.tile(
                                [128, NT], F32, tag="facc", bufs=1, name="facc"
                            )
                        if FP8_OUTPROJ:
                            nc.tensor.matmul(
                                facc,
                                wo_sb[:, :, dc, :],
                                outn_sb[:, :, ds(q0, NT)],
                                start=True,
                                stop=True,
                                perf_mode=DR,
                            )
                        else:
                            for h in range(NH):
                                nc.tensor.matmul(
                                    facc,
                                    wo_sb[:, h, dc, :],
                                    outn_sb[:, h, ds(q0, NT)],
                                    start=(h == 0),
                                    stop=(h == NH - 1),
                                )
                        fsb = fop.tile([128, NT], F32, tag="f")
                        if FP8_OUTPROJ:
                            nc.vector.tensor_scalar_mul(
                                fsb, facc, 1.0 / (WSC * WSC)
                            )
                        elif dc % 2:
                            # split evacs across engines so the next sweep's
                            # mask-multiplies don't queue behind them on DVE
                            nc.scalar.activation(
                                out=fsb, in_=facc, func=Copy, bias=0.0, scale=1.0
                            )
                        else:
                            nc.vector.tensor_copy(fsb, facc)
                        qB.dma_start(
                            out=outT[dc * 128 : (dc + 1) * 128, ds(q0, NT)], in_=fsb
                        )

    return nc


_NC_CACHE = None


def _get_nc():
    global _NC_CACHE
    if _NC_CACHE is None:
        nc = build()
        nc.compile()
        _NC_CACHE = nc
    return _NC_CACHE


def make_in_maps(q, k, v, mask, Wq, bq, Wk, bk, Wv, bv, Wo, bo):
    bf = ml_dtypes.bfloat16
    f8 = ml_dtypes.float8_e4m3
    q = np.asarray(q, np.float32)
    k = np.asarray(k, np.float32)
    v = np.asarray(v, np.float32)
    mask = np.asarray(mask)
    Ws = [np.asarray(w, np.float32) for w in (Wq, Wk, Wv)]
    bs = [np.asarray(b_, np.float32) for b_ in (bq, bk, bv)]
    # q/k biases don't commute with the device layout; v bias folds into bo
    # on the host (softmax weights sum to 1). This model family has zeros.
    assert not np.any(bs[0]) and not np.any(bs[1]), "nonzero q/k bias unsupported"
    Wo = np.asarray(Wo, np.float32)

    # x packed per batch: [3, kc(p), 128, (2,) S]
    xPb = []
    for b in range(B):
        xs = np.stack([q[b].T, k[b].T, v[b].T])  # [3, DM, S]
        xp_ = xs.reshape(3, KCP, 2, 128, S).transpose(0, 1, 3, 2, 4)
        if FP8_PROJ:
            xPb.append(np.ascontiguousarray(np.clip(xp_, -240, 240)).astype(f8))
        else:
            xPb.append(np.ascontiguousarray(xp_).astype(bf))

    # mask packed per batch: [half, j, p, oc, 512]; maskT[key, query]
    maskPb = []
    for b in range(B):
        mT = mask[b].T.astype(np.float32)  # [key, query]
        mP = mT.reshape(OC, 128, 2, 2, NT).transpose(2, 3, 1, 0, 4)
        maskPb.append(np.ascontiguousarray(mP).astype(bf))

    # W[dm, dout] with head h owning columns d*H+h
    Wr = [W.reshape(KC, 128, D, H) for W in Ws]  # [kc, p, d, h]
    br = [b_.reshape(D, H) for b_ in bs]
    ident = np.eye(128, dtype=np.float32).astype(bf)

    in_maps = []
    for c in range(8):
        b = c // 4
        h0 = NH * (c % 4)
        if FP8_PROJ:
            w_core = np.empty((128, 3, KCP, 2, NH, D), np.float32)
            for t in range(3):
                for hi in range(NH):
                    # w_core[p, t, kcp, i, hi, d] = Wr[t][kcp*2+i, p, d, h0+hi]
                    w_core[:, t, :, :, hi, :] = (
                        Wr[t][:, :, :, h0 + hi]
                        .reshape(KCP, 2, 128, D)
                        .transpose(2, 0, 1, 3)
                    )
            w_core = np.clip(w_core * WSC, -240, 240).astype(f8)
        else:
            w_core = np.empty((128, 3, KC, NH, D), np.float32)
            for t in range(3):
                for hi in range(NH):
                    w_core[:, t, :, hi, :] = Wr[t][:, :, :, h0 + hi].transpose(
                        1, 0, 2
                    )
            w_core = w_core.astype(bf)
        # wo[d, h, dc, 128]: head h0+hi rows of Wo (rows h::H), cols chunked
        wo_core = np.empty((D, NH, DOUT // 128, 128), np.float32)
        for hi in range(NH):
            wo_core[:, hi] = Wo[h0 + hi :: H, :].reshape(D, DOUT // 128, 128)
        if FP8_OUTPROJ:
            wo_core = np.clip(wo_core * WSC, -240, 240).astype(f8)
        else:
            wo_core = wo_core.astype(bf)
        in_maps.append(
            {
                "x": xPb[b],
                "maskP": maskPb[b],
                "ident": ident,
                "w": np.ascontiguousarray(w_core),
                "wo": np.ascontiguousarray(wo_core),
            }
        )
    return in_maps


def unshard(results, bo=None):
    out = np.zeros((B, DOUT, S), np.float32)
    for c in range(8):
        out[c // 4] += np.asarray(results[c]["outT"], np.float32)
    out = out.transpose(0, 2, 1)  # [B, S, DOUT]
    if bo is not None:
        out = out + np.asarray(bo, np.float32)[None, None, :]
    return np.ascontiguousarray(out)


def kernel(**inputs):
    in_maps = make_in_maps(**inputs)
    nc = _get_nc()
    res = run_bass_kernel_spmd(nc, in_maps, core_ids=list(range(8)))
    bo_eff = np.asarray(inputs["bo"], np.float32) + np.asarray(
        inputs["bv"], np.float32
    ) @ np.asarray(inputs["Wo"], np.float32)
    return unshard(res.results, bo=bo_eff)
